# revision 57
# baseline (speedup 1.0000x reference)
"""DGCNN (nn_DGCNN_50594714747409) Bass/TRN2 kernel — 8-core data parallel.

Contract: kernel(**inputs) takes the FULL unsharded inputs (as produced by
setup_inputs()) and returns the FULL [16, 101] output. Internally shards the
batch (16) across 8 NeuronCores (2 samples/core), runs one SPMD Bass program
per core via bass_utils.run_bass_kernel_spmd, and concatenates the outputs.

Algorithm notes (refactor of the reference; error budget rel<2e-2):
  * EdgeConv: max_k(lrelu(bn(W @ [x_j - x_i; x_i]))) with bn scale > 0 and
    lrelu monotone ==> lrelu(bn(max_k(Wn@x_j) + (Wc-Wn)@x_i)). BN folded into
    conv weights on the host.
  * kNN: top-20 of s[i,j] = x_i.x_j - 0.5||x_j||^2 (same per-row order as the
    reference's -||x_i - x_j||^2). Distance matmuls run in f32r on the PE.
    Top-20 selection via int32 packing (M<<10)+j so the DVE needs only 5
    passes (3x max8 + 2x match_replace) over the packed values bitcast to
    f32 (nonneg IEEE floats order like their bit patterns); indices come
    from the low 10 bits - no max_index scans. The pack itself never
    touches the DVE (PACK_V2): Act quantizes d*S*1024 + 3*2^32 (f32 ulp
    1024 in [2^33,2^34) zeroes the low 10 bits), a second Act op shifts
    down to exact i32 multiples of 1024, and the Pool engine adds iota
    (int32 add == bitwise-or here; Pool has no TSPtr/bitwise on HW).
  * The whole value path is fp16: a = Wn@x stored fp16 in HBM (rows padded
    to 256B for O=64 layers), SWDGE dma_gather (two i-tiles per gather for
    O<=128), and the 20-way neighbor max as an in-place fp16 TT-max chain
    on the DVE - all-16-bit tensor_tensor hits the DVE 2x perf mode while
    tensor_reduce never does. Distances/features stay f32.
  * Bias + LeakyReLU fused into the output transpose: PE accumulates
    (Wb@x)^T + z^T in PSUM, Act engine evicts with Prelu(alpha=0.2) and the
    per-channel bias br as the per-partition activation bias.
  * conv5 (weights preloaded to SBUF) + fused global max/mean pool (mean
    via activation accum_out); FC head batched, FC1 in fp16.
  * Software-pipelined emission at i-tile granularity: each sample's
    gather/chain phase 2 is interleaved step-by-step with the OTHER
    sample's distance/top-K phase 1 (generators + merged driver), and the
    next layer's preamble (fp16 copy, column norms, quantization scale) is
    emitted chunk-by-chunk inside the previous gather phase. Engine queues
    are in-order, so emission order is per-engine execution order.
  * Known HW pitfalls honored: TensorScalarPtr and bitwise int32 ops are
    DVE-only; Pool TT max fails codegen; gpsimd XYZWC reduce reads all 128
    partitions (unusable for 1-partition tiles); CoreSim models Pool int
    adds in f32 (HW is exact).
"""

import numpy as np

import concourse.bass as bass
import concourse.bacc as bacc
import concourse.mybir as mybir
from concourse.tile import TileContext
from concourse import bass_utils

F32 = mybir.dt.float32
F32R = mybir.dt.float32r
F16 = mybir.dt.float16
U32 = mybir.dt.uint32
I32 = mybir.dt.int32
I16 = mybir.dt.int16
ALU = mybir.AluOpType
ACTF = mybir.ActivationFunctionType

N = 1024
KNN = 20
EPS = 1e-5
LAYERS = [(3, 64), (64, 64), (64, 128), (128, 256)]  # (C_in, O)
NCORES = 8
BPC = 2  # samples per core
PACK_V2 = True  # A1/A2 magic-rounding pack (Act+Pool) vs DVE TSPtr pack
MAGIC = float(2 ** 23)  # f32 magic: y = d*S + 2^23 + 2^19 rounds to int M


def build_nc(bpc=BPC):
    nc = bacc.Bacc("TRN2", target_bir_lowering=False, debug=False)

    # ---- I/O ----
    x_in = nc.dram_tensor("x", [bpc, 3, N], F32R, kind="ExternalInput")
    win = {}
    for l, (C, O) in enumerate(LAYERS, 1):
        win[f"wnt{l}"] = nc.dram_tensor(f"wnt{l}", [C, O], F16, kind="ExternalInput")
        win[f"wbt{l}"] = nc.dram_tensor(f"wbt{l}", [C, O], F16, kind="ExternalInput")
        win[f"brt{l}"] = nc.dram_tensor(f"brt{l}", [O, 1], F32, kind="ExternalInput")
    win["w5t"] = nc.dram_tensor("w5t", [4, 128, N], F32R, kind="ExternalInput")
    win["b5c"] = nc.dram_tensor("b5c", [128, 8], F32, kind="ExternalInput")
    win["w6t"] = nc.dram_tensor("w6t", [16, 128, 512], F16, kind="ExternalInput")
    win["b6r"] = nc.dram_tensor("b6r", [1, 512], F32R, kind="ExternalInput")
    win["w7t"] = nc.dram_tensor("w7t", [4, 128, 256], F32R, kind="ExternalInput")
    win["b7r"] = nc.dram_tensor("b7r", [1, 256], F32R, kind="ExternalInput")
    win["w8t"] = nc.dram_tensor("w8t", [2, 128, 101], F32, kind="ExternalInput")
    win["b8r"] = nc.dram_tensor("b8r", [1, 101], F32, kind="ExternalInput")
    ident_in = nc.dram_tensor("ident", [128, 128], F32, kind="ExternalInput")
    ident16_in = nc.dram_tensor("ident16", [128, 128], F16, kind="ExternalInput")
    ones_in = nc.dram_tensor("ones", [1, 128], F32R, kind="ExternalInput")
    mhalf_in = nc.dram_tensor("mhalf", [128, 1], F32R, kind="ExternalInput")
    iota_in = nc.dram_tensor("iota", [128, N], I32, kind="ExternalInput")
    qb_in = nc.dram_tensor("qb", [128, 1], F32, kind="ExternalInput")
    qb2_in = nc.dram_tensor("qb2", [128, 1], F32, kind="ExternalInput")
    out = nc.dram_tensor("out", [bpc, 101], F32, kind="ExternalOutput")

    with TileContext(nc) as tc:
        import contextlib
        ctx = contextlib.ExitStack()
        with ctx:
            wpool = ctx.enter_context(tc.tile_pool(name="w", bufs=1))
            pool = ctx.enter_context(tc.tile_pool(name="sb", bufs=2))
            big = ctx.enter_context(tc.tile_pool(name="big", bufs=1))
            psum = ctx.enter_context(tc.tile_pool(name="ps", bufs=2, space="PSUM"))
            dram = ctx.enter_context(tc.tile_pool(name="dr", bufs=2, space="DRAM"))

            # ---- stage weights/constants into SBUF ----
            ident = wpool.tile([128, 128], F32, tag="ident")
            nc.sync.dma_start(ident[:], ident_in[:])
            ident16 = wpool.tile([128, 128], F16, tag="ident16")
            nc.sync.dma_start(ident16[:], ident16_in[:])
            ones = wpool.tile([1, 128], F32R, tag="ones")
            nc.sync.dma_start(ones[:], ones_in[:])
            mhalf = wpool.tile([128, 1], F32R, tag="mhalf")
            nc.sync.dma_start(mhalf[:], mhalf_in[:])
            iota = wpool.tile([128, N], I32, tag="iota")
            nc.sync.dma_start(iota[:], iota_in[:])
            qb = wpool.tile([128, 1], F32, tag="qb")
            nc.sync.dma_start(qb[:], qb_in[:])
            qb2 = wpool.tile([128, 1], F32, tag="qb2")
            nc.sync.dma_start(qb2[:], qb2_in[:])
            wsb = {}
            for l, (C, O) in enumerate(LAYERS, 1):
                wsb[f"wnt{l}"] = wpool.tile([C, O], F16, tag=f"wnt{l}", name=f"wnt{l}")
                wsb[f"wbt{l}"] = wpool.tile([C, O], F16, tag=f"wbt{l}", name=f"wbt{l}")
                hb = (O + 127) // 128
                wsb[f"brt{l}"] = wpool.tile([min(O, 128), hb], F32,
                                            tag=f"brt{l}", name=f"brt{l}")
                nc.sync.dma_start(wsb[f"wnt{l}"][:], win[f"wnt{l}"][:])
                nc.sync.dma_start(wsb[f"wbt{l}"][:], win[f"wbt{l}"][:])
                nc.sync.dma_start(
                    wsb[f"brt{l}"][:],
                    win[f"brt{l}"][:].rearrange("(h p) o -> p (h o)", h=hb))
            for k, shp, kdt in [("b5c", [128, 8], F32),
                                ("b6r", [1, 512], F32R),
                                ("w7t", [128, 4, 256], F32R),
                                ("b7r", [1, 256], F32R),
                                ("w8t", [128, 2, 101], F32),
                                ("b8r", [1, 101], F32)]:
                wsb[k] = wpool.tile(shp, kdt, tag=k, name=k)
                if len(shp) == 3:
                    nc.sync.dma_start(wsb[k][:], win[k][:].rearrange("a b c -> b a c"))
                else:
                    nc.sync.dma_start(wsb[k][:], win[k][:])

            # per-sample persistent feature tiles (xc = concat of layer outputs)
            # xc[s][0]: ch 0-127 (x1 | x2), xc[s][1]: x3, xc[s][2:4]: x4
            xc = [[big.tile([128, N], F32R, tag=f"xc{s}_{t}", name=f"xc{s}_{t}")
                   for t in range(4)] for s in range(bpc)]
            x2own = [big.tile([64, N], F32R, tag=f"x2own{s}", name=f"x2own{s}")
                     for s in range(bpc)]
            p2 = big.tile([128, 16, bpc], F16, tag="p2")  # pooled [max|mean]

            def make_pre(s, l, cur, C):
                """Per-layer preamble state: fp16 feature copy + column
                norms + quantization scale. emit_chunk(it) handles one
                128-column chunk so the whole preamble can be emitted
                inside the PREVIOUS layer's phase-2 loop (as output columns
                land), hiding its serial ramp; finish() derives the scale."""
                t = f"{s}"
                cur16 = pool.tile([C, N], F16, tag="cur16" + t, bufs=2)
                sq = pool.tile([C, N], F32R, tag="sq" + t, bufs=1)
                negxx = pool.tile([1, N], F32R, tag="negxx" + t)
                m0 = pool.tile([1, 4], F32, tag="m0" + t)
                s_sb = pool.tile([128, 1], F32, tag="ssb" + t)

                def emit_chunk(it):
                    isl = slice(it * 128, (it + 1) * 128)
                    nc.scalar.copy(cur16[:, isl], cur[:, isl])
                    nc.scalar.activation(sq[:, isl], cur[:, isl], ACTF.Square)
                    xx_ps = psum.tile([1, 128], F32, tag="aux")
                    nc.tensor.matmul(xx_ps[:], mhalf[:C, :], sq[:, isl],
                                     start=True, stop=True)
                    nc.scalar.copy(negxx[:, isl], xx_ps[:])

                def finish():
                    # NOTE: gpsimd XYZWC reduce is NOT safe here -- it
                    # reads the full 128-partition SBUF region but negxx is
                    # a 1-partition tile (partitions 1-127 belong to other
                    # tiles), which poisons the scale. Keep the min on DVE.
                    nc.vector.tensor_reduce(m0[:, 0:1], negxx[:].bitcast(F32),
                                            axis=mybir.AxisListType.X,
                                            op=ALU.min)
                    # m0 = -0.5*max||x||^2 (<= 0)
                    nc.vector.tensor_scalar(out=m0[:, 1:2], in0=m0[:, 0:1],
                                            scalar1=-1.0, scalar2=1e-12,
                                            op0=ALU.mult, op1=ALU.add)
                    nc.vector.reciprocal(m0[:, 2:3], m0[:, 1:2])
                    nc.vector.tensor_scalar(out=m0[:, 3:4], in0=m0[:, 2:3],
                                            scalar1=float(2 ** 27 if PACK_V2
                                                          else 2 ** 17),
                                            scalar2=None, op0=ALU.mult)
                    s_ps = psum.tile([128, 1], F32, tag="aux")
                    nc.tensor.matmul(s_ps[:], ones[:, :].bitcast(F32),
                                     m0[:, 3:4], start=True, stop=True)
                    nc.scalar.copy(s_sb[:], s_ps[:])

                return {"cur": cur, "cur16": cur16, "negxx": negxx,
                        "s_sb": s_sb, "emit_chunk": emit_chunk,
                        "finish": finish}

            def edge_layer(s, l, pre, C, O, dests):
                """pre: make_pre state (already emitted). dests[h]: list of
                (tile, row_off) for 128-row chunk h of the [O, N] output.

                Emits phase 1 (distance + top-K + aT/index staging) and
                RETURNS a closure that emits phase 2 (gather + neighbor max
                + fused output). Callers stagger the two phases across the
                two samples so DMA-bound phase 2 overlaps the other
                sample's DVE-bound phase 1."""
                t = f"{s}"
                wnt, wbt = wsb[f"wnt{l}"], wsb[f"wbt{l}"]
                brt = wsb[f"brt{l}"]
                cur = pre["cur"]
                cur16 = pre["cur16"]
                negxx = pre["negxx"]
                s_sb = pre["s_sb"]

                idx = pool.tile([128, 8, 24], F32, tag="idx" + t)
                # SWDGE gather needs elem_size % 256 bytes: for O=64 layers
                # the fp16 row is padded to 128 cols (same 256B/descriptor as
                # the old f32 path) so every layer gets the f16 TT-max chain.
                adt = F16
                aelem = max(O, 128)
                at_dr = dram.tile([N, aelem], adt, tag="at_dr" + t)
                idxw_all = pool.tile([KNN, 8, 128], I16, tag="idxw" + t, bufs=1)
                jA = dram.tile([16, 1280], I16, tag="jA" + t)
                jAv = jA[:].rearrange("r (it t g) -> r it t g", it=8, t=KNN, g=8)
                jsb = pool.tile([128, 1280], I16, tag="jsb" + t, bufs=1)

                def p1_step(it):
                    isl = slice(it * 128, (it + 1) * 128)
                    d_ps = psum.tile([128, N], F32, tag="dist")
                    for jc in range(2):
                        jsl = slice(jc * 512, (jc + 1) * 512)
                        nc.tensor.matmul(d_ps[:, jsl], cur[:, isl],
                                         cur[:, jsl], start=True, stop=False)
                        nc.tensor.matmul(d_ps[:, jsl], ones[:, :128],
                                         negxx[:, jsl], start=False, stop=True)
                    # Pack (M<<10)+j without touching the DVE:
                    #  A1 (Act): q1f = d*S*1024 + 3*2^32. The whole range
                    #     sits in [2^33, 2^34) where f32 ulp is 1024, so the
                    #     low 10 bits of the integer value are forced to 0.
                    #  A2 (Act): dsb = i32(q1f - 3*2^32 + 2^29) -- exact
                    #     (multiples of 1024, < 2^30), nonnegative.
                    #  Pool: dsb += iota (int32 add == bitwise or here; the
                    #     Pool engine supports int add but not bitwise/TSPtr).
                    dsb = pool.tile([128, N], I32, tag="dsb" + t)
                    if PACK_V2:
                        q1f = pool.tile([128, N], F32, tag="q1f", bufs=2)
                        nc.scalar.activation(q1f[:], d_ps[:], ACTF.Identity,
                                             bias=qb[:, 0:1], scale=s_sb[:, 0:1])
                        nc.scalar.activation(dsb[:], q1f[:], ACTF.Identity,
                                             bias=qb2[:, 0:1], scale=1.0)
                        nc.gpsimd.tensor_tensor(dsb[:], dsb[:], iota[:],
                                                op=ALU.add)
                    else:
                        nc.scalar.activation(dsb[:], d_ps[:], ACTF.Identity,
                                             bias=qb[:, 0:1], scale=s_sb[:, 0:1])
                        eng = nc.vector
                        eng.add_instruction(mybir.InstTensorScalarPtr(
                            name=eng.bass.get_next_instruction_name(),
                            is_scalar_tensor_tensor=True,
                            op0=ALU.arith_shift_left, op1=ALU.bitwise_or,
                            ins=[eng.lower_ap(dsb[:]),
                                 mybir.ImmediateValue(dtype=I32, value=10),
                                 eng.lower_ap(iota[:])],
                            outs=[eng.lower_ap(dsb[:])]))
                    dsf = dsb[:].bitcast(F32)
                    # exact top-24 >= top-20 per row: 3 rounds of max8
                    mx = pool.tile([128, 24], F32, tag="mx" + t)
                    for r in range(3):
                        nc.vector.max(mx[:, r * 8:(r + 1) * 8], dsf)
                        if r < 2:
                            nc.vector.match_replace(dsf, mx[:, r * 8:(r + 1) * 8],
                                                    dsf, 0.0)
                    # idx = packed & 1023 (low 10 bits); the F32 output
                    # value-converts the small int result so the PE
                    # transpose can consume it directly (no copy)
                    nc.vector.tensor_scalar(
                        out=idx[:, it, :], in0=mx[:].bitcast(U32),
                        scalar1=1023, scalar2=None, op0=ALU.bitwise_and)
                    # aT (fp16, to HBM for the gather)
                    a_ps = psum.tile([128, O], F32, tag="aux")
                    nc.tensor.matmul(a_ps[:], cur16[:, isl], wnt[:],
                                     start=True, stop=True)
                    a_st = pool.tile([128, aelem], adt, tag="a_st" + t)
                    nc.scalar.copy(a_st[:, 0:O], a_ps[:])
                    if aelem > O:
                        # duplicate into the pad so the DMA never reads
                        # uninitialized SBUF (values are ignored downstream)
                        nc.scalar.copy(a_st[:, O:2 * O], a_ps[:])
                    nc.sync.dma_start(at_dr[isl, :], a_st[:])
                    # J wrap: jA[r, it*160 + t*8 + g] = idx[16g+r, it, t]
                    it_ps = psum.tile([KNN, 128], F32, tag="aux")
                    nc.tensor.transpose(it_ps[:], idx[:, it, 0:KNN], ident[:])
                    wv = idxw_all[:, it, :].rearrange("t (r g) -> t r g", r=16, g=8)
                    sv = it_ps[:].rearrange("t (g r) -> t r g", g=8, r=16)
                    nc.scalar.copy(wv, sv)
                    nc.sync.dma_start(
                        jAv[:, it, :, :].rearrange("r t g -> t r g"),
                        idxw_all[:, it, :].rearrange("t (r g) -> t r g",
                                                     r=16, g=8))

                def p1_steps():
                    for it in range(8):
                        p1_step(it)
                        yield
                    for gg in range(8):
                        nc.sync.dma_start(jsb[16 * gg:16 * (gg + 1), :], jA[:])

                def p2_steps(next_pre=None):
                    return _edge_phase2(s, l, cur16, jsb, at_dr, adt, aelem,
                                        O, wbt, brt, dests, next_pre)
                return p1_steps, p2_steps

            def _edge_phase2(s, l, cur16, jsb, at_dr, adt, aelem, O, wbt,
                             brt, dests, next_pre):
                # gather (fp16) + 20-way max reduce per i-tile, fused with
                # out = lrelu(z^T + (Wb@cur) + br): PE accumulates bT + z^T
                # in PSUM, Act evicts with Prelu and the per-channel bias.
                # For O <= 128 two i-tiles share one SWDGE gather (fewer Pool
                # descriptor-gen slots; the shared "gath" tag is already
                # sized for layer 4 so this costs no extra SBUF).
                nper = 2 if aelem <= 128 else 1
                g_t = None
                for it in range(8):
                    isl = slice(it * 128, (it + 1) * 128)
                    if it % nper == 0:
                        g_t = pool.tile([128, nper * KNN, aelem], adt,
                                        tag="gath", bufs=2)
                        jslice = jsb[:, it * 160:(it + nper) * 160]
                        nc.gpsimd.dma_gather(
                            out_ap=g_t[:], in_ap=at_dr[:], idxs_ap=jslice,
                            num_idxs=nper * KNN * 128,
                            num_idxs_reg=nper * KNN * 128,
                            elem_size=aelem, single_packet=False)
                    toff = (it % nper) * KNN
                    gv = g_t[:, toff:toff + KNN, 0:O]
                    z = pool.tile([128, O], F32, tag="z", bufs=2)
                    if True:
                        # 20-way neighbor max as an in-place f16 TT-max chain:
                        # all-16-bit packed operands hit the DVE 2x perf mode
                        # (tensor_reduce never does). The final step emits f32
                        # so the PE transpose below keeps its f32 path.
                        nc.vector.tensor_tensor(gv[:, 0:10, :], gv[:, 0:10, :],
                                                gv[:, 10:20, :], op=ALU.max)
                        nc.vector.tensor_tensor(gv[:, 0:5, :], gv[:, 0:5, :],
                                                gv[:, 5:10, :], op=ALU.max)
                        nc.vector.tensor_tensor(gv[:, 0:2, :], gv[:, 0:2, :],
                                                gv[:, 2:4, :], op=ALU.max)
                        nc.vector.tensor_tensor(gv[:, 0:1, :], gv[:, 0:1, :],
                                                gv[:, 1:2, :], op=ALU.max)
                        nc.vector.tensor_tensor(z[:], gv[:, 0, :],
                                                gv[:, 4, :], op=ALU.max)
                    for h in range((O + 127) // 128):
                        oc = min(128, O - h * 128)
                        hsl = slice(h * 128, h * 128 + oc)
                        t_ps = psum.tile([128, 128], F32, tag="fc")
                        nc.tensor.matmul(t_ps[:oc, :], wbt[:, hsl],
                                         cur16[:, isl], start=True, stop=False)
                        nc.tensor.matmul(
                            t_ps[:oc, :], z[:, hsl],
                            ident[:], is_transpose=True, start=False, stop=True)
                        for dt_, roff in dests[h]:
                            nc.scalar.activation(
                                dt_[roff:roff + oc, it * 128:(it + 1) * 128],
                                t_ps[:oc, :], ACTF.Prelu,
                                bias=brt[0:oc, h:h + 1], alpha=0.2)
                    if next_pre is not None:
                        next_pre["emit_chunk"](it)
                    yield
                if next_pre is not None:
                    next_pre["finish"]()

            # preload conv5/FC1 weights once (overlaps the edge layers)
            w5sb = wpool.tile([128, 4, N], F32R, tag="w5sb")
            nc.sync.dma_start(w5sb[:], win["w5t"][:].rearrange("a b c -> b a c"))
            w6sb = wpool.tile([128, 16, 512], F16, tag="w6sb")
            nc.sync.dma_start(w6sb[:], win["w6t"][:].rearrange("t p c -> p t c"))

            def conv5_steps(s):
                # conv5 (f32r) + fused global max/mean pool
                accs = pool.tile([128, 8, 2], F32, tag=f"accs{s}")
                pmax = pool.tile([128, 8, 2], F32, tag=f"pmax{s}")
                for m in range(8):
                    yield
                    for c in range(2):
                        y_ps = psum.tile([128, 512], F32, tag="dist")
                        for kt in range(4):
                            nc.tensor.matmul(
                                y_ps[:], w5sb[:, kt, m * 128:(m + 1) * 128],
                                xc[s][kt][:, c * 512:(c + 1) * 512],
                                start=(kt == 0), stop=(kt == 3))
                        z5 = pool.tile([128, 512], F32, tag=f"z5{s}")
                        nc.scalar.activation(z5[:], y_ps[:], ACTF.Prelu,
                                             bias=wsb["b5c"][:, m:m + 1],
                                             alpha=0.2,
                                             accum_out=accs[:, m, c:c + 1])
                        nc.vector.tensor_reduce(
                            pmax[:, m, c:c + 1], z5[:],
                            axis=mybir.AxisListType.X, op=ALU.max)
                nc.vector.tensor_tensor(p2[:, 0:8, s], pmax[:, :, 0],
                                        pmax[:, :, 1], op=ALU.max)
                asm = pool.tile([128, 8], F32, tag=f"asm{s}")
                nc.vector.tensor_tensor(asm[:], accs[:, :, 0], accs[:, :, 1],
                                        op=ALU.add)
                nc.vector.tensor_scalar(
                    out=p2[:, 8:16, s], in0=asm[:], scalar1=1.0 / N,
                    scalar2=None, op0=ALU.mult)

            LCFG = {1: (3, 64), 2: (64, 64), 3: (64, 128), 4: (128, 256)}

            def cur_ap(s, l):
                if l == 2:
                    return xc[s][0][0:64, :]
                if l == 3:
                    return x2own[s][:]
                return xc[s][1][:]

            def dests_of(s, l):
                if l == 1:
                    return [[(xc[s][0], 0)]]
                if l == 2:
                    return [[(xc[s][0], 64), (x2own[s], 0)]]
                if l == 3:
                    return [[(xc[s][1], 0)]]
                return [[(xc[s][2], 0)], [(xc[s][3], 0)]]

            def emit_p1(s, l, pre):
                C, O = LCFG[l]
                return edge_layer(s, l, pre, C, O, dests_of(s, l))

            # Software-pipelined emission. In-order engine queues make the
            # emission order the execution order per engine, so the sequence
            # is chosen so that (a) each phase-1 preamble (Act/Pool/PE
            # serial ramp) is covered by the other sample's DVE work, and
            # (b) each sample's gather chains (DVE) directly follow its own
            # scans, never stuck behind the other sample's next scans.
            #   P1(0,1) P1(1,1) then per layer: P2(0,l) P1(0,l+1) P2(1,l)
            #   P1(1,l+1); at l=4 the P1 slots become conv5(s).
            def drain(g):
                for _ in g:
                    pass

            def merged(p2g, p1g_):
                # interleave per i-tile: gathers/chains (p2) lead, the other
                # sample's distance/top-K (p1) fills between them
                while True:
                    a = next(p2g, _SENTINEL)
                    b = next(p1g_, _SENTINEL)
                    if a is _SENTINEL and b is _SENTINEL:
                        break

            _SENTINEL = object()
            pres, p1g, p2f = {}, {}, {}
            for s_ in range(bpc):
                x_sb = pool.tile([3, N], F32R, tag=f"x_in{s_}", bufs=1)
                nc.sync.dma_start(x_sb[:], x_in[s_])
                pres[s_] = make_pre(s_, 1, x_sb[:], 3)
                for it_ in range(8):
                    pres[s_]["emit_chunk"](it_)
                pres[s_]["finish"]()
                g_, f_ = emit_p1(s_, 1, pres[s_])
                p1g[s_], p2f[s_] = g_(), f_
            drain(p1g[0])                    # P1(0,1) stands alone
            prev_p1 = p1g[1]
            rounds = [(ss, ll) for ll in range(1, 5) for ss in range(bpc)]
            for i, (s_, l_) in enumerate(rounds):
                np_ = (make_pre(s_, l_ + 1, cur_ap(s_, l_ + 1),
                                LCFG[l_ + 1][0]) if l_ < 4 else None)
                merged(p2f[s_](np_), prev_p1)
                if l_ < 4:
                    g_, f_ = emit_p1(s_, l_ + 1, np_)
                    p2f[s_] = f_
                    prev_p1 = g_()
                else:
                    prev_p1 = conv5_steps(s_)

            # ---- FC head (both samples batched) ----
            h1_ps = psum.tile([bpc, 512], F32, tag="fc")
            for tt in range(16):
                nc.tensor.matmul(h1_ps[:], p2[:, tt, :],
                                 w6sb[:, tt, :], start=(tt == 0), stop=False)
            nc.tensor.matmul(h1_ps[:], ones[:, 0:bpc].bitcast(F32R),
                             wsb["b6r"][:].bitcast(F32R), start=False, stop=True)
            h1 = pool.tile([bpc, 512], F32, tag="h1")
            nc.scalar.copy(h1[:], h1_ps[:])
            nc.vector.scalar_tensor_tensor(out=h1[:], in0=h1[:], scalar=0.2,
                                           in1=h1[:], op0=ALU.mult, op1=ALU.max)
            h1t = pool.tile([128, 4, bpc], F32R, tag="h1t")
            for kt in range(4):
                t_ps = psum.tile([128, bpc], F32, tag="aux")
                nc.tensor.transpose(t_ps[:], h1[:, kt * 128:(kt + 1) * 128],
                                    ident[0:bpc, 0:bpc])
                nc.scalar.copy(h1t[:, kt, :], t_ps[:])
            h2_ps = psum.tile([bpc, 256], F32, tag="fc")
            for t in range(4):
                nc.tensor.matmul(h2_ps[:], h1t[:, t, :].bitcast(F32R),
                                 wsb["w7t"][:, t, :],
                                 start=(t == 0), stop=False)
            nc.tensor.matmul(h2_ps[:], ones[:, 0:bpc],
                             wsb["b7r"][:], start=False, stop=True)
            h2 = pool.tile([bpc, 256], F32, tag="h2")
            nc.scalar.copy(h2[:], h2_ps[:])
            nc.vector.scalar_tensor_tensor(out=h2[:], in0=h2[:], scalar=0.2,
                                           in1=h2[:], op0=ALU.mult, op1=ALU.max)
            h2t = pool.tile([128, 2, bpc], F32, tag="h2t")
            for kt in range(2):
                t_ps = psum.tile([128, bpc], F32, tag="aux")
                nc.tensor.transpose(t_ps[:], h2[:, kt * 128:(kt + 1) * 128],
                                    ident[0:bpc, 0:bpc])
                nc.scalar.copy(h2t[:, kt, :], t_ps[:])
            o_ps = psum.tile([bpc, 101], F32, tag="fc")
            for t in range(2):
                nc.tensor.matmul(o_ps[:], h2t[:, t, :], wsb["w8t"][:, t, 0:101],
                                 start=(t == 0), stop=False)
            nc.tensor.matmul(o_ps[:], ones[:, 0:bpc].bitcast(F32), wsb["b8r"][:],
                             start=False, stop=True)
            o_sb = pool.tile([bpc, 101], F32, tag="osb")
            nc.scalar.copy(o_sb[:], o_ps[:])
            nc.sync.dma_start(out[:], o_sb[:])

    nc.finalize()
    return nc


def prep_weights(inp):
    """Host-side: fold BN into weights; device-friendly layouts."""
    d = {}
    f32 = np.float32
    f16 = np.float16
    for l, (C, O) in enumerate(LAYERS, 1):
        w = np.asarray(inp[f"w{l}"], f32)
        g, b = np.asarray(inp[f"g{l}"], f32), np.asarray(inp[f"b{l}"], f32)
        m, v = np.asarray(inp[f"m{l}"], f32), np.asarray(inp[f"v{l}"], f32)
        s = g / np.sqrt(v + EPS)
        assert (s > 0).all(), "BN scale must be positive for the max/act swap"
        wn = w[:, :C] * s[:, None]
        wb = (w[:, C:] - w[:, :C]) * s[:, None]
        d[f"wnt{l}"] = np.ascontiguousarray(wn.T, f16)
        d[f"wbt{l}"] = np.ascontiguousarray(wb.T, f16)
        d[f"brt{l}"] = np.ascontiguousarray((b - m * s)[:, None], f32)
    s5 = np.asarray(inp["g5"], f32) / np.sqrt(np.asarray(inp["v5"], f32) + EPS)
    assert (s5 > 0).all()
    d["w5t"] = np.ascontiguousarray(
        (np.asarray(inp["w5"], f32) * s5[:, None]).T.reshape(4, 128, N), f32)
    d["b5c"] = np.ascontiguousarray(
        (np.asarray(inp["b5"], f32) - np.asarray(inp["m5"], f32) * s5)
        .reshape(8, 128).T, f32)
    s6 = np.asarray(inp["g6"], f32) / np.sqrt(np.asarray(inp["v6"], f32) + EPS)
    d["w6t"] = np.ascontiguousarray(
        (np.asarray(inp["wl1"], f32) * s6[:, None]).T.reshape(16, 128, 512), f16)
    d["b6r"] = np.ascontiguousarray(
        (np.asarray(inp["b6"], f32) - np.asarray(inp["m6"], f32) * s6)[None, :], f32)
    s7 = np.asarray(inp["g7"], f32) / np.sqrt(np.asarray(inp["v7"], f32) + EPS)
    d["w7t"] = np.ascontiguousarray(
        (np.asarray(inp["wl2"], f32) * s7[:, None]).T.reshape(4, 128, 256), f32)
    d["b7r"] = np.ascontiguousarray(
        (s7 * (np.asarray(inp["bl2"], f32) - np.asarray(inp["m7"], f32))
         + np.asarray(inp["b7"], f32))[None, :], f32)
    d["w8t"] = np.ascontiguousarray(
        np.asarray(inp["wl3"], f32).T.reshape(2, 128, 101), f32)
    d["b8r"] = np.ascontiguousarray(np.asarray(inp["bl3"], f32)[None, :], f32)
    d["ident"] = np.eye(128, dtype=f32)
    d["ident16"] = np.eye(128, dtype=f16).copy()
    d["ones"] = np.ones((1, 128), f32)
    d["mhalf"] = np.full((128, 1), -0.5, f32)
    d["iota"] = np.broadcast_to(np.arange(N, dtype=np.int32)[None, :],
                                (128, N)).copy()
    d["qb"] = np.full((128, 1), 3.0 * 2.0 ** 32 if PACK_V2 else 2.0 ** 19, f32)
    d["qb2"] = np.full((128, 1), -(3.0 * 2.0 ** 32) + 2.0 ** 29, f32)
    return d


_CACHE = {}


def _get_nc():
    if "nc" not in _CACHE:
        _CACHE["nc"] = build_nc()
    return _CACHE["nc"]


def kernel(**inputs):
    x = np.ascontiguousarray(np.asarray(inputs["x"], np.float32))
    assert x.shape == (16, 3, N), x.shape
    prep = prep_weights(inputs)
    nc = _get_nc()
    in_maps = []
    for c in range(NCORES):
        m = dict(prep)
        m["x"] = np.ascontiguousarray(x[c * BPC:(c + 1) * BPC])
        in_maps.append(m)
    res = bass_utils.run_bass_kernel_spmd(nc, in_maps, core_ids=list(range(NCORES)))
    out = np.concatenate([r["out"] for r in res.results], axis=0)
    return out.astype(np.float32)

